# revision 3
# baseline (speedup 1.0000x reference)
"""Trainium2 Bass kernel for the AttentionBlock problem.

Full inputs:  x [16, 64, 64, 64] f32, w_theta [8, 64], w_phi [8, 64],
              w_g [32, 64], w_o [64, 32], gamma [] (all f32).
Sharding: data-parallel over batch, 2 samples per core on 8 NeuronCores.

Per-sample math (C=64, S=4096, T=S/4=1024):
  theta = w_theta @ x            [8, S]
  phi   = pool2x2(w_phi @ x)     [8, T]
  g     = pool2x2(w_g @ x)       [32, T]
  scoresT[t, s] = sum_c phi[c, t] theta[c, s]
  expT = exp(scoresT)            (no max-subtraction; |scores| <~ 20 is fp32-safe)
  attnU[c, s] = sum_t g[c, t] expT[t, s];  Z[s] = sum_t expT[t, s]
     (one matmul: lhsT = gT' [t, g(32) | ones(32)] so rows 32:64 of the
      output are Z broadcast across 32 partitions)
  attnS = attnU / Z
  oT[s, co] = sum_c attnS[c, s] * (gamma * w_o[co, c])   (gamma folded on host)
  out[c, s] = transpose(oT) + x
"""

import sys

if "/opt/trn_rl_repo" not in sys.path:
    sys.path.insert(0, "/opt/trn_rl_repo")

import numpy as np

import concourse.bass as bass
import concourse.tile as tile
from concourse import bacc, mybir
from concourse.bass_utils import run_bass_kernel_spmd

F32 = mybir.dt.float32
AF = mybir.ActivationFunctionType

B, C, H, W = 16, 64, 64, 64
S = H * W            # 4096
T = S // 4           # 1024
NCORES = 8
BLOC = B // NCORES   # 2 samples per core
NT = T // 128        # 8 t-tiles
CHUNK = 1024         # s-chunk size
NCH = S // CHUNK     # 4 chunks per sample


def _build_sample(nc, tc, pools, s, x_ext, out_ext, wct_sb, wog_sb, ident_sb):
    (pp_sc, pp_at, pp_sm, p_samp, p_chunk, p_sub) = pools

    # ---- load x: [64, 4096] -> SBUF [128, 2048]; partition p = 64*a + c
    # holds x[c, a*2048 : (a+1)*2048]
    x_sb = p_samp.tile([128, 2048], F32, tag="x_sb")
    nc.sync.dma_start(x_sb[0:64, :], x_ext[s, :, 0:2048])
    nc.sync.dma_start(x_sb[64:128, :], x_ext[s, :, 2048:4096])

    # ---- fused 1x1 convs: [48, 512] psum chunks -> tpg_sb [48, 4096]
    # rows 0:8 theta, 32:40 phi(unpooled), 64:96 g(unpooled) (32-aligned bases)
    tpg_sb = p_samp.tile([96, 4096], F32, tag="tpg_sb")
    for k in range(8):
        a = k // 4
        ps_conv = pp_sm.tile([96, 512], F32, tag="sm", name=f"ps_conv_{s}_{k}")
        nc.tensor.matmul(
            ps_conv[:],
            wct_sb[64 * a:64 * a + 64, :],
            x_sb[64 * a:64 * a + 64, (k % 4) * 512:(k % 4) * 512 + 512],
            start=True, stop=True,
        )
        nc.vector.tensor_copy(tpg_sb[:, k * 512:(k + 1) * 512], ps_conv[:])

    # ---- maxpool 2x2 for phi and g (w-pairs then h-pairs), via strided APs
    phi_sb = p_samp.tile([8, T], F32, tag="phi_sb")
    g_sb = p_samp.tile([32, T], F32, tag="g_sb")
    pw_sb = p_samp.tile([8, 2048], F32, tag="pw_sb")
    gw_sb = p_samp.tile([32, 2048], F32, tag="gw_sb")

    def pool_w(dst, src):
        # src [p, 4096] viewed [p, 2048, 2]; dst [p, 2048]
        sv = src.rearrange("p (x two) -> p x two", two=2)
        dv = dst.rearrange("p (x one) -> p x one", one=1)
        nc.vector.tensor_max(dv, sv[:, :, 0:1], sv[:, :, 1:2])

    def pool_h(dst, src):
        # src [p, 2048] viewed [p, 32, 2, 32] (h = 2q+r, w2 32); dst [p, 1024]
        sv = src.rearrange("p (q r w) -> p q r w", r=2, w=32)
        dv = dst.rearrange("p (q one w) -> p q one w", one=1, w=32)
        nc.vector.tensor_max(dv, sv[:, :, 0:1, :], sv[:, :, 1:2, :])

    pool_w(pw_sb[:], tpg_sb[32:40, :])
    pool_h(phi_sb[:], pw_sb[:])
    pool_w(gw_sb[:], tpg_sb[64:96, :])
    pool_h(g_sb[:], gw_sb[:])

    # ---- gT' tiles: [128, 64] per t-tile = [g block transposed | ones]
    gT_sb = p_samp.tile([128, NT * 64], F32, tag="gT_sb")
    nc.vector.memset(gT_sb[:], 1.0)
    for t in range(NT):
        ps_tr = pp_sm.tile([128, 32], F32, tag="sm", name=f"ps_gtr_{s}_{t}")
        nc.tensor.transpose(
            ps_tr[:], g_sb[:, t * 128:(t + 1) * 128], ident_sb[0:32, 0:32]
        )
        nc.vector.tensor_copy(gT_sb[:, t * 64:t * 64 + 32], ps_tr[:])

    theta = tpg_sb[0:8, :]

    # ---- attention, per s-chunk
    for ch in range(NCH):
        expT = p_chunk.tile([128, NT * CHUNK], F32, tag="expT")
        for t in range(NT):
            ps_sc = pp_sc.tile([128, CHUNK], F32, tag="sc", name=f"ps_sc_{s}_{ch}_{t}")
            for h in range(CHUNK // 512):
                nc.tensor.matmul(
                    ps_sc[:, h * 512:(h + 1) * 512],
                    phi_sb[:, t * 128:(t + 1) * 128],
                    theta[:, ch * CHUNK + h * 512:ch * CHUNK + (h + 1) * 512],
                    start=True, stop=True,
                )
            nc.scalar.activation(
                expT[:, t * CHUNK:(t + 1) * CHUNK], ps_sc[:], AF.Exp
            )

        ps_at = pp_at.tile([64, CHUNK], F32, tag="at", name=f"ps_at_{s}_{ch}")
        for t in range(NT):
            for h in range(CHUNK // 512):
                nc.tensor.matmul(
                    ps_at[:, h * 512:(h + 1) * 512],
                    gT_sb[:, t * 64:(t + 1) * 64],
                    expT[:, t * CHUNK + h * 512:t * CHUNK + (h + 1) * 512],
                    start=(t == 0), stop=(t == NT - 1),
                )

        rz_sb = p_chunk.tile([32, CHUNK], F32, tag="rz_sb")
        attnS = p_chunk.tile([32, CHUNK], F32, tag="attnS")
        nc.vector.reciprocal(rz_sb[:], ps_at[32:64, :])
        nc.vector.tensor_mul(attnS[:], ps_at[0:32, :], rz_sb[:])

        out_sb = p_chunk.tile([64, CHUNK], F32, tag="out_sb")
        for j in range(CHUNK // 128):
            ps_oT = pp_sm.tile([128, 64], F32, tag="sm", name=f"ps_oT_{s}_{ch}_{j}")
            nc.tensor.matmul(
                ps_oT[:], attnS[:, j * 128:(j + 1) * 128], wog_sb[:],
                start=True, stop=True,
            )
            oT_sb = p_sub.tile([128, 64], F32, tag="oT_sb")
            nc.vector.tensor_copy(oT_sb[:], ps_oT[:])
            ps_fin = pp_sm.tile([64, 128], F32, tag="sm", name=f"ps_fin_{s}_{ch}_{j}")
            nc.tensor.transpose(ps_fin[:], oT_sb[:], ident_sb[:])
            st = ch * (CHUNK // 128) + j        # global subtile 0..31
            a, b_off = st // 16, (st % 16) * 128
            nc.vector.tensor_add(
                out_sb[:, j * 128:(j + 1) * 128],
                ps_fin[:],
                x_sb[64 * a:64 * a + 64, b_off:b_off + 128],
            )
        nc.sync.dma_start(out_ext[s, :, ch * CHUNK:(ch + 1) * CHUNK], out_sb[:])


def build_nc():
    nc = bacc.Bacc("TRN2", target_bir_lowering=False, debug=False,
                   num_devices=NCORES)
    x_ext = nc.dram_tensor("x", [BLOC, C, S], F32, kind="ExternalInput").ap()
    wct_ext = nc.dram_tensor("wct", [128, 96], F32, kind="ExternalInput").ap()
    wog_ext = nc.dram_tensor("wog", [32, 64], F32, kind="ExternalInput").ap()
    ident_ext = nc.dram_tensor("ident", [128, 128], F32, kind="ExternalInput").ap()
    out_ext = nc.dram_tensor("out", [BLOC, C, S], F32, kind="ExternalOutput").ap()

    with tile.TileContext(nc) as tc:
        with (
            tc.tile_pool(name="wpool", bufs=1) as p_w,
            tc.tile_pool(name="samp", bufs=2) as p_samp,
            tc.tile_pool(name="chunk", bufs=2) as p_chunk,
            tc.tile_pool(name="sub", bufs=2) as p_sub,
            tc.tile_pool(name="ppsc", bufs=2, space="PSUM") as pp_sc,
            tc.tile_pool(name="ppat", bufs=1, space="PSUM") as pp_at,
            tc.tile_pool(name="ppsm", bufs=2, space="PSUM") as pp_sm,
        ):
            wct_sb = p_w.tile([128, 96], F32, tag="wct_sb")
            wog_sb = p_w.tile([32, 64], F32, tag="wog_sb")
            ident_sb = p_w.tile([128, 128], F32, tag="ident_sb")
            nc.sync.dma_start(wct_sb[:], wct_ext[:])
            nc.sync.dma_start(wog_sb[:], wog_ext[:])
            nc.sync.dma_start(ident_sb[:], ident_ext[:])

            pools = (pp_sc, pp_at, pp_sm, p_samp, p_chunk, p_sub)
            for s in range(BLOC):
                _build_sample(nc, tc, pools, s, x_ext, out_ext,
                              wct_sb, wog_sb, ident_sb)

    nc.compile()
    return nc


_NC_CACHE = None


def _get_nc():
    global _NC_CACHE
    if _NC_CACHE is None:
        _NC_CACHE = build_nc()
    return _NC_CACHE


def kernel(x, w_theta, w_phi, w_g, w_o, gamma):
    x = np.ascontiguousarray(np.asarray(x, dtype=np.float32))
    w_theta = np.asarray(w_theta, dtype=np.float32)
    w_phi = np.asarray(w_phi, dtype=np.float32)
    w_g = np.asarray(w_g, dtype=np.float32)
    w_o = np.asarray(w_o, dtype=np.float32)
    gamma_f = float(np.asarray(gamma, dtype=np.float32))

    # lhsT for the fused conv: [64, 48] = [w_theta.T | w_phi.T | w_g.T],
    # replicated on partitions 64:128 (conv rhs for the second x half
    # lives at base partition 64, and lhsT rows must align with rhs rows).
    wcat = np.zeros((64, 96), dtype=np.float32)
    wcat[:, 0:8] = w_theta.T
    wcat[:, 32:40] = w_phi.T
    wcat[:, 64:96] = w_g.T
    wct = np.tile(wcat, (2, 1))                        # [128, 96]
    wog = (gamma_f * w_o).T.astype(np.float32)         # [32, 64]
    wog = np.ascontiguousarray(wog)
    ident = np.eye(128, dtype=np.float32)

    nc = _get_nc()
    xr = x.reshape(B, C, S)
    in_maps = [
        {
            "x": np.ascontiguousarray(xr[i * BLOC:(i + 1) * BLOC]),
            "wct": wct,
            "wog": wog,
            "ident": ident,
        }
        for i in range(NCORES)
    ]
    res = run_bass_kernel_spmd(nc, in_maps, core_ids=list(range(NCORES)))
    out = np.concatenate([res.results[i]["out"] for i in range(NCORES)], axis=0)
    return out.reshape(B, C, H, W).astype(np.float32)


if __name__ == "__main__":
    rng = np.random.default_rng(0)
    ins = {
        "x": rng.standard_normal((B, C, H, W), dtype=np.float32),
        "w_theta": (rng.standard_normal((8, 64)) / 8.0).astype(np.float32),
        "w_phi": (rng.standard_normal((8, 64)) / 8.0).astype(np.float32),
        "w_g": (rng.standard_normal((32, 64)) / 8.0).astype(np.float32),
        "w_o": (rng.standard_normal((64, 32)) / np.sqrt(32)).astype(np.float32),
        "gamma": np.float32(0.7),
    }
    out = kernel(**ins)
    print("out", out.shape, out.dtype, np.abs(out).mean())


# revision 9
# speedup vs baseline: 1.4112x; 1.4112x over previous
"""Trainium2 Bass kernel for the AttentionBlock problem.

Full inputs:  x [16, 64, 64, 64] f32, w_theta [8, 64], w_phi [8, 64],
              w_g [32, 64], w_o [64, 32], gamma [] (all f32).
Sharding: data-parallel over batch, 2 samples per core on 8 NeuronCores.

Per-sample math (C=64, S=4096, T=S/4=1024):
  theta = w_theta @ x            [8, S]
  phi   = pool2x2(w_phi @ x)     [8, T]
  g     = pool2x2(w_g @ x)       [32, T]
  scoresT[t, s] = sum_c phi[c, t] theta[c, s]
  expT = exp(scoresT)            (no max-subtraction; |scores| <~ 20 is fp32-safe)
  attnU[c, s] = sum_t g[c, t] expT[t, s];  Z[s] = sum_t expT[t, s]
     (one matmul: lhsT = gT' [t, g(32) | ones(32)] so rows 32:64 of the
      output are Z broadcast across 32 partitions)
  attnS = attnU / Z
  o = (gamma * w_o) @ attnS      (gamma folded on host)
  out = o + x

Matmul operands are typed float32r (TF32-speed PE path, 4-byte layout).
"""

import sys

if "/opt/trn_rl_repo" not in sys.path:
    sys.path.insert(0, "/opt/trn_rl_repo")

import numpy as np

import concourse.bass as bass
import concourse.tile as tile
from concourse import bacc, mybir
from concourse.bass_utils import run_bass_kernel_spmd

F32 = mybir.dt.float32
F32R = mybir.dt.float32r
AF = mybir.ActivationFunctionType

B, C, H, W = 16, 64, 64, 64
S = H * W            # 4096
T = S // 4           # 1024
NCORES = 8
BLOC = B // NCORES   # 2 samples per core
NT = T // 128        # 8 t-tiles
CHUNK = 1024         # s-chunk size
NCH = S // CHUNK     # 4 chunks per sample


def _build_sample(nc, tc, pools, s, x_ext, out_ext, wct_sb, wog_sb, ident_sb,
                  gtinit_ext):
    (pp_sc, pp_at, pp_sm, p_samp, p_chunk) = pools

    # ---- load x: [64, 4096] -> SBUF [128, 2048]; partition p = 64*a + c
    # holds x[c, a*2048 : (a+1)*2048]
    x_sb = p_samp.tile([128, 2048], F32R, tag="x_sb")
    nc.sync.dma_start(x_sb[0:64, :], x_ext[s, :, 0:2048])
    nc.sync.dma_start(x_sb[64:128, :], x_ext[s, :, 2048:4096])

    # ---- fused 1x1 convs: [96, 512] psum chunks -> tpg_sb [96, 4096]
    # rows 0:8 theta, 32:40 phi(unpooled), 64:96 g(unpooled) (32-aligned bases)
    tpg_sb = p_samp.tile([96, 4096], F32R, tag="tpg_sb")
    for k in range(8):
        a = k // 4
        ps_conv = pp_sm.tile([96, 512], F32, tag="sm", name=f"ps_conv_{s}_{k}")
        nc.tensor.matmul(
            ps_conv[:],
            wct_sb[64 * a:64 * a + 64, :],
            x_sb[64 * a:64 * a + 64, (k % 4) * 512:(k % 4) * 512 + 512],
            start=True, stop=True,
        )
        nc.vector.tensor_copy(tpg_sb[:, k * 512:(k + 1) * 512], ps_conv[:])

    # ---- maxpool 2x2 for phi and g (w-pairs then h-pairs), via strided APs
    phi_sb = p_samp.tile([8, T], F32R, tag="phi_sb")
    g_sb = p_samp.tile([32, T], F32R, tag="g_sb")
    pw_sb = p_samp.tile([8, 2048], F32R, tag="pw_sb")
    gw_sb = p_samp.tile([32, 2048], F32R, tag="gw_sb")

    def pool_w(dst, src):
        # src [p, 4096] viewed [p, 2048, 2]; dst [p, 2048]
        sv = src.rearrange("p (x two) -> p x two", two=2)
        dv = dst.rearrange("p (x one) -> p x one", one=1)
        nc.vector.tensor_max(dv, sv[:, :, 0:1], sv[:, :, 1:2])

    def pool_h(dst, src):
        # src [p, 2048] viewed [p, 32, 2, 32] (h = 2q+r, w2 32); dst [p, 1024]
        sv = src.rearrange("p (q r w) -> p q r w", r=2, w=32)
        dv = dst.rearrange("p (q one w) -> p q one w", one=1, w=32)
        nc.vector.tensor_max(dv, sv[:, :, 0:1, :], sv[:, :, 1:2, :])

    pool_w(pw_sb[:], tpg_sb[32:40, :])
    pool_h(phi_sb[:], pw_sb[:])
    pool_w(gw_sb[:], tpg_sb[64:96, :])
    pool_h(g_sb[:], gw_sb[:])

    # ---- gT' tiles: [128, 64] per t-tile = [g block transposed | ones]
    gT_sb = p_samp.tile([128, NT * 64], F32R, tag="gT_sb")
    nc.sync.dma_start(gT_sb[:], gtinit_ext[:])
    for t in range(NT):
        ps_tr = pp_sm.tile([128, 32], F32R, tag="sm", name=f"ps_gtr_{s}_{t}")
        nc.tensor.transpose(
            ps_tr[:], g_sb[:, t * 128:(t + 1) * 128], ident_sb[0:32, 0:32]
        )
        nc.vector.tensor_copy(gT_sb[:, t * 64:t * 64 + 32], ps_tr[:])

    theta = tpg_sb[0:8, :]

    # ---- attention, per s-chunk
    for ch in range(NCH):
        expT = p_chunk.tile([128, NT * CHUNK], F32R, tag="expT")
        for t in range(NT):
            ps_sc = pp_sc.tile([128, CHUNK], F32, tag="sc", name=f"ps_sc_{s}_{ch}_{t}")
            for h in range(CHUNK // 512):
                nc.tensor.matmul(
                    ps_sc[:, h * 512:(h + 1) * 512],
                    phi_sb[:, t * 128:(t + 1) * 128],
                    theta[:, ch * CHUNK + h * 512:ch * CHUNK + (h + 1) * 512],
                    start=True, stop=True,
                )
            nc.scalar.activation(
                expT[:, t * CHUNK:(t + 1) * CHUNK], ps_sc[:], AF.Exp
            )

        ps_at = pp_at.tile([64, CHUNK], F32, tag="at", name=f"ps_at_{s}_{ch}")
        for t in range(NT):
            for h in range(CHUNK // 512):
                nc.tensor.matmul(
                    ps_at[:, h * 512:(h + 1) * 512],
                    gT_sb[:, t * 64:(t + 1) * 64],
                    expT[:, t * CHUNK + h * 512:t * CHUNK + (h + 1) * 512],
                    start=(t == 0), stop=(t == NT - 1),
                )

        rz_sb = p_chunk.tile([32, CHUNK], F32, tag="rz_sb")
        attnS = p_chunk.tile([32, CHUNK], F32R, tag="attnS")
        nc.vector.reciprocal(rz_sb[:], ps_at[32:64, :])
        nc.vector.tensor_mul(attnS[:], ps_at[0:32, :], rz_sb[:])

        out_sb = p_chunk.tile([64, CHUNK], F32, tag="out_sb")
        for h in range(CHUNK // 512):
            ps_o = pp_sm.tile([64, 512], F32, tag="sm", name=f"ps_o_{s}_{ch}_{h}")
            nc.tensor.matmul(
                ps_o[:], wog_sb[:], attnS[:, h * 512:(h + 1) * 512],
                start=True, stop=True,
            )
            s0 = ch * CHUNK + h * 512           # global s offset
            a, b_off = s0 // 2048, s0 % 2048
            nc.vector.tensor_add(
                out_sb[:, h * 512:(h + 1) * 512],
                ps_o[:],
                x_sb[64 * a:64 * a + 64, b_off:b_off + 512].bitcast(F32),
            )
        nc.sync.dma_start(out_ext[s, :, ch * CHUNK:(ch + 1) * CHUNK], out_sb[:])


def build_nc():
    nc = bacc.Bacc("TRN2", target_bir_lowering=False, debug=False,
                   num_devices=NCORES)
    x_ext = nc.dram_tensor("x", [BLOC, C, S], F32R, kind="ExternalInput").ap()
    wct_ext = nc.dram_tensor("wct", [128, 96], F32R, kind="ExternalInput").ap()
    wog_ext = nc.dram_tensor("wog", [32, 64], F32R, kind="ExternalInput").ap()
    ident_ext = nc.dram_tensor("ident", [128, 128], F32R, kind="ExternalInput").ap()
    gtinit_ext = nc.dram_tensor("gtinit", [128, NT * 64], F32R,
                                kind="ExternalInput").ap()
    out_ext = nc.dram_tensor("out", [BLOC, C, S], F32, kind="ExternalOutput").ap()

    with tile.TileContext(nc) as tc:
        with (
            tc.tile_pool(name="wpool", bufs=1) as p_w,
            tc.tile_pool(name="samp", bufs=2) as p_samp,
            tc.tile_pool(name="chunk", bufs=2) as p_chunk,
            tc.tile_pool(name="ppsc", bufs=2, space="PSUM") as pp_sc,
            tc.tile_pool(name="ppat", bufs=1, space="PSUM") as pp_at,
            tc.tile_pool(name="ppsm", bufs=2, space="PSUM") as pp_sm,
        ):
            wct_sb = p_w.tile([128, 96], F32R, tag="wct_sb")
            wog_sb = p_w.tile([32, 64], F32R, tag="wog_sb")
            ident_sb = p_w.tile([128, 128], F32R, tag="ident_sb")
            nc.sync.dma_start(wct_sb[:], wct_ext[:])
            nc.sync.dma_start(wog_sb[:], wog_ext[:])
            nc.sync.dma_start(ident_sb[:], ident_ext[:])

            pools = (pp_sc, pp_at, pp_sm, p_samp, p_chunk)
            for s in range(BLOC):
                _build_sample(nc, tc, pools, s, x_ext, out_ext,
                              wct_sb, wog_sb, ident_sb, gtinit_ext)

    nc.compile()
    return nc


_NC_CACHE = None


def _get_nc():
    global _NC_CACHE
    if _NC_CACHE is None:
        _NC_CACHE = build_nc()
    return _NC_CACHE


def kernel(x, w_theta, w_phi, w_g, w_o, gamma):
    x = np.ascontiguousarray(np.asarray(x, dtype=np.float32))
    w_theta = np.asarray(w_theta, dtype=np.float32)
    w_phi = np.asarray(w_phi, dtype=np.float32)
    w_g = np.asarray(w_g, dtype=np.float32)
    w_o = np.asarray(w_o, dtype=np.float32)
    gamma_f = float(np.asarray(gamma, dtype=np.float32))

    # lhsT for the fused conv: [64, 96] = [w_theta.T | pad | w_phi.T | pad |
    # w_g.T] (phi at col 32, g at col 64 so SBUF partition bases stay
    # 32-aligned), replicated on partitions 64:128 (conv rhs for the second
    # x half lives at base partition 64; lhsT rows must align with rhs rows).
    wcat = np.zeros((64, 96), dtype=np.float32)
    wcat[:, 0:8] = w_theta.T
    wcat[:, 32:40] = w_phi.T
    wcat[:, 64:96] = w_g.T
    wct = np.tile(wcat, (2, 1))                        # [128, 96]
    wog = np.ascontiguousarray((gamma_f * w_o).T)      # [32, 64]
    ident = np.eye(128, dtype=np.float32)
    gtinit = np.zeros((128, NT * 64), dtype=np.float32)
    for t in range(NT):
        gtinit[:, t * 64 + 32:t * 64 + 64] = 1.0

    nc = _get_nc()
    xr = x.reshape(B, C, S)
    in_maps = [
        {
            "x": np.ascontiguousarray(xr[i * BLOC:(i + 1) * BLOC]),
            "wct": wct,
            "wog": wog,
            "ident": ident,
            "gtinit": gtinit,
        }
        for i in range(NCORES)
    ]
    res = run_bass_kernel_spmd(nc, in_maps, core_ids=list(range(NCORES)))
    out = np.concatenate([res.results[i]["out"] for i in range(NCORES)], axis=0)
    return out.reshape(B, C, H, W).astype(np.float32)


if __name__ == "__main__":
    rng = np.random.default_rng(0)
    ins = {
        "x": rng.standard_normal((B, C, H, W), dtype=np.float32),
        "w_theta": (rng.standard_normal((8, 64)) / 8.0).astype(np.float32),
        "w_phi": (rng.standard_normal((8, 64)) / 8.0).astype(np.float32),
        "w_g": (rng.standard_normal((32, 64)) / 8.0).astype(np.float32),
        "w_o": (rng.standard_normal((64, 32)) / np.sqrt(32)).astype(np.float32),
        "gamma": np.float32(0.7),
    }
    out = kernel(**ins)
    print("out", out.shape, out.dtype, np.abs(out).mean())
